# revision 62
# baseline (speedup 1.0000x reference)
"""MultiHeadAttention Trainium2 kernel (8 NeuronCores, SPMD), v2.

Problem: B=2, L=2048, DK=DV=512, H=8, dh=64.
  Q = q @ WQ[h]; K = k @ WK[h]; V = v @ WV[h]       (per head)
  y = Q K^T / sqrt(L); z = softmax(y, axis=QUERY); out = z @ V
  concat heads on feature dim.

Sharding: 16 (b,h) pairs over 8 cores -> 2 heads (same batch) per core.

v2 vs v1 (81.3us -> 74.6us).  Engine busy went ACT 62.2->49.4,
DVE 57.8->49.9, PE 51.8->42.0; the wall is warmup-DMA lead + the
saturated, balanced exp/evac loop + the AV replay tail.
  * Scores matmuls run in fp8e4m3 with perf_mode=DoubleRow at 0.5
    cycles/row.  The 64-deep head contraction is packed to the required
    2-per-partition pair layout with stride-0 broadcast APs (each
    feature streamed twice -> computes 2*y; the 1/2 folds into the exp
    scale).  Q/K projections evacuate straight to fp8 (~+0.7% rel err,
    budget-checked).
  * Score PSUM tiles are [128, 1024] (2 banks); exp runs as halves,
    amortizing ACT's fixed access-latency + accum-read costs.  THREE
    score slots rotate so neither exp engine ever waits on a PSUM
    refill round-trip (2 slots measured +~0.7us/kt of exposed latency).
  * h0 exp exact on ACT (fused accum D); h1 on DVE Schraudolph
    (bf16-bits, whole blocks so the bias cancels in E/D) with a
    4x-mode copy-accum D pass.  Per-head stat chains are emitted so no
    engine's in-order queue fences on another engine's unfinished work
    (h0's D chain closes one iteration late when its deps are stale).
  * The 8 PSUM banks can't hold 3 score slots AND a 16-q-tile AV
    accumulator: the loop accumulates q-tiles 0-7 in 2 banks; q-tiles
    8-15 replay over the all-live E tiles in a PE-only tail into a
    fresh accumulator carved from the then-idle score pool, with the
    evacs overlapped bank-by-bank.
  * V projection batched 4 k-tiles per PSUM tile, one f32 evac; Pool's
    normalize_recip does Vs = Vf*(1/D) in one op (no DVE reciprocals).
  * kt0 scores at 512-wide quarters so the first exps start as soon as
    the first q/k projection spans land; dummy PE matmuls bridge the
    input-DMA wait so the HAM p-state ramp completes before real work.
"""

import math

import numpy as np

B = 2
L = 2048
DK = 512
H = 8
DH = 64
P = 128
NKT = L // P  # 16 k-tiles
NDC = DK // P  # 4 d-chunks
N_CORES = 8

SCALE = 1.0 / math.sqrt(float(L))
# Schraudolph in bf16-bits domain: round(raw*EXP_A2 + EXP_B) as int16,
# bitcast bf16 ~= exp(raw*SCALE).  Raw fp8 scores are 2*y so EXP_A2
# carries a 1/2.
EXP_A2 = 128.0 * math.log2(math.e) * SCALE * 0.5
EXP_B = 16256.0 - 12.0
SCALE2 = SCALE * 0.5  # ACT exp scale on the doubled raw scores

# schedule knobs (swept via TimelineSim)
CFG = dict(
    w_queue="scalar",  # weight DMA issue queue: 'sync' | 'scalar'
    dma_variant=0,  # 0: baseline order, 1: wq,wk first
    warm_qp1="dve",  # engine for warmup q-quarter-1 evac
    warm_qp23="dve",  # engine for kt0 q-quarter-2/3 evacs
    kproj_eng="act",  # engine for kproj quarter evacs (kts 1-3)
    av0_pos="after_vproj",  # 'before_kproj' | 'after_vproj'
    av1_pos="after_early",  # 'after_early' | 'after_f1'
    vf_eng="act",  # 'act' | 'dve' | 'split'
    scr_bufs=2,
    vf_bufs=3,
    prewarm_mms=36,
)

_CACHE = {}


def _build_program(cfg=None):
    import concourse.bass as bass
    import concourse.tile as tile
    from concourse import bacc, mybir
    from concourse.bass import ts

    c = dict(CFG)
    if cfg:
        c.update(cfg)

    f32 = mybir.dt.float32
    bf16 = mybir.dt.bfloat16
    fp8 = mybir.dt.float8e4
    i16 = mybir.dt.int16
    AF = mybir.ActivationFunctionType
    ALU = mybir.AluOpType
    DR = mybir.MatmulPerfMode.DoubleRow

    nc = bacc.Bacc("TRN2", target_bir_lowering=False, debug=False)

    qt_d = nc.dram_tensor("qt", [DK, L], bf16, kind="ExternalInput")
    kt_d = nc.dram_tensor("kt", [DK, L], bf16, kind="ExternalInput")
    vt_d = nc.dram_tensor("vt", [DK, L], bf16, kind="ExternalInput")
    wq_d = nc.dram_tensor("wq", [DK, P], bf16, kind="ExternalInput")
    wk_d = nc.dram_tensor("wk", [DK, P], bf16, kind="ExternalInput")
    wv_d = nc.dram_tensor("wv", [DK, P], bf16, kind="ExternalInput")
    out_d = nc.dram_tensor("out", [P, NKT, P], bf16, kind="ExternalOutput")

    with tile.TileContext(nc) as tc:
        with (
            tc.tile_pool(name="consts", bufs=1) as consts,
            tc.tile_pool(name="xin", bufs=1) as xin,
            tc.tile_pool(name="proj", bufs=1) as proj,
            tc.tile_pool(name="epool", bufs=2 * NKT) as epool,
            tc.tile_pool(name="vfpool", bufs=c["vf_bufs"]) as vfpool,
            tc.tile_pool(name="scrpool", bufs=c["scr_bufs"]) as scrpool,
            tc.tile_pool(name="vspool", bufs=NKT) as vspool,
            tc.tile_pool(name="stats", bufs=1) as stats,
            tc.tile_pool(name="outp", bufs=1) as outp,
            tc.tile_pool(name="spsum", bufs=3, space="PSUM") as spsum,
            tc.tile_pool(name="avpsum", bufs=1, space="PSUM") as avpsum,
        ):
            wq_s = consts.tile([P, NDC, P], bf16)
            wk_s = consts.tile([P, NDC, P], bf16)
            wv_s = consts.tile([P, NDC, P], bf16)
            qt_s = xin.tile([P, NDC, L], bf16)
            kt_s = xin.tile([P, NDC, L], bf16)
            vt_s = xin.tile([P, NDC, L], bf16)
            qt_r = qt_d.rearrange("(o p) l -> p o l", p=P)
            kt_r = kt_d.rearrange("(o p) l -> p o l", p=P)
            vt_r = vt_d.rearrange("(o p) l -> p o l", p=P)

            def load_chunk(sb, rr, ch):
                nc.sync.dma_start(sb[:, :, ts(ch, 512)], rr[:, :, ts(ch, 512)])

            def load_w(sb, rr):
                r = rr.rearrange("(o p) e -> p o e", p=P)
                if c["w_queue"] == "scalar":
                    nc.scalar.dma_start(sb[:], r)
                else:
                    nc.sync.dma_start(sb[:], r)

            # PE p-state: dummy matmuls from t~0 keep the PE busy through
            # the input-DMA wait so the 3.4us HAM ramp completes (and
            # pe_busy_start never resets) before the first projection
            zt = consts.tile([P, P], bf16)
            nc.gpsimd.memset(zt[:], 0.0)
            pwp = spsum.tile([P, 1024], f32, tag="sco", name="prewarm")
            for _ in range(c["prewarm_mms"]):
                nc.tensor.matmul(
                    pwp[:, 0:P], lhsT=zt[:], rhs=zt[:], start=True, stop=True
                )

            if c["dma_variant"] == 0:
                # k0a is a tiny 128-col K slice so the kp_mini projection
                # (and with it kt0's first score quarters) starts ~4us
                # before the full k-chunk lands
                load_w(wq_s, wq_d)
                load_chunk(qt_s, qt_r, 0)
                load_w(wk_s, wk_d)
                load_chunk(kt_s, kt_r, 0)
                load_chunk(qt_s, qt_r, 1)
                load_chunk(qt_s, qt_r, 2)
                load_chunk(qt_s, qt_r, 3)
                load_w(wv_s, wv_d)
                load_chunk(vt_s, vt_r, 0)
                load_chunk(kt_s, kt_r, 1)
                load_chunk(kt_s, kt_r, 2)
                load_chunk(vt_s, vt_r, 1)
                load_chunk(kt_s, kt_r, 3)
                load_chunk(vt_s, vt_r, 2)
                load_chunk(vt_s, vt_r, 3)
            else:
                load_w(wq_s, wq_d)
                load_chunk(qt_s, qt_r, 0)
                load_w(wk_s, wk_d)
                load_chunk(kt_s, kt_r, 0)
                load_chunk(qt_s, qt_r, 1)
                load_chunk(qt_s, qt_r, 2)
                load_chunk(qt_s, qt_r, 3)
                load_w(wv_s, wv_d)
                load_chunk(vt_s, vt_r, 0)
                load_chunk(kt_s, kt_r, 1)
                load_chunk(kt_s, kt_r, 2)
                load_chunk(vt_s, vt_r, 1)
                load_chunk(kt_s, kt_r, 3)
                load_chunk(vt_s, vt_r, 2)
                load_chunk(vt_s, vt_r, 3)

            # fp8 projected Q/K: head h features on partitions h*64..h*64+64
            QT8 = proj.tile([P, L], fp8)
            KT8 = proj.tile([P, L], fp8)

            Dsum2 = stats.tile([P, NKT, 2, 2], f32)  # ACT accum halves
            Dtot = stats.tile([P, NKT * 2], f32)
            Dq4 = stats.tile([P, 4], f32)  # kt0 quarter accums (h0)
            Dq2 = stats.tile([P, 2], f32)

            # AV accumulator: 2 banks hold q-tiles 0-7 during the kt loop;
            # q-tiles 8-15 replay over the all-live E tiles in a PE tail
            # that reuses the same banks after the first-half evac
            avpA = avpsum.tile([P, 8, P], f32, tag="ava")

            def qk_proj_span(W, X, OUT8, lo, width, eng):
                # project cols [lo, lo+width): gated only on the input
                # chunks covering that span
                ps = spsum.tile([P, 1024], f32, tag="sco", name="qkproj")
                for dc in range(NDC):
                    nc.tensor.matmul(
                        ps[:, 0:width],
                        lhsT=W[:, dc, :],
                        rhs=X[:, dc, lo : lo + width],
                        start=(dc == 0),
                        stop=(dc == NDC - 1),
                    )
                if eng == "act":
                    nc.scalar.copy(OUT8[:, lo : lo + width], ps[:, 0:width])
                else:
                    nc.vector.tensor_copy(OUT8[:, lo : lo + width], ps[:, 0:width])

            Etiles = {}
            Vstiles = {}
            Vfinfo = {}

            def scores_quarter_kt0(h, qc, E):
                # kt0 runs at 512-wide quarters so the first exps start as
                # soon as each q-projection span lands
                hp = h * DH
                ps = spsum.tile([P, 1024], f32, tag="sco", name="sco0")
                lhs = KT8[hp : hp + DH, 0:P].unsqueeze(1).broadcast_to([DH, 2, P])
                rhs = (
                    QT8[hp : hp + DH, ts(qc, 512)]
                    .unsqueeze(1)
                    .broadcast_to([DH, 2, 512])
                )
                nc.tensor.matmul(
                    ps[:, 0:512], lhsT=lhs, rhs=rhs, start=True, stop=True,
                    perf_mode=DR,
                )
                if h == 0:
                    nc.scalar.activation(
                        E[:, ts(qc, 512)],
                        ps[:, 0:512],
                        AF.Exp,
                        scale=SCALE2,
                        accum_out=Dq4[:, qc : qc + 1],
                    )
                else:
                    e16 = E[:, ts(qc, 512)].bitcast(i16)
                    nc.vector.tensor_scalar(
                        e16, ps[:, 0:512], EXP_A2, EXP_B, ALU.mult, ALU.add
                    )

            def scores_half(kt, h, hf, E):
                # one [128, 1024] half: 2 DoubleRow fp8 matmuls + one exp
                hp = h * DH
                ps = spsum.tile([P, 1024], f32, tag="sco", name="sco")
                for sub in range(2):
                    qc = hf * 2 + sub
                    lhs = (
                        KT8[hp : hp + DH, ts(kt, P)]
                        .unsqueeze(1)
                        .broadcast_to([DH, 2, P])
                    )
                    rhs = (
                        QT8[hp : hp + DH, ts(qc, 512)]
                        .unsqueeze(1)
                        .broadcast_to([DH, 2, 512])
                    )
                    nc.tensor.matmul(
                        ps[:, ts(sub, 512)],
                        lhsT=lhs,
                        rhs=rhs,
                        start=True,
                        stop=True,
                        perf_mode=DR,
                    )
                if h == 0:
                    nc.scalar.activation(
                        E[:, ts(hf, 1024)],
                        ps[:],
                        AF.Exp,
                        scale=SCALE2,
                        accum_out=Dsum2[:, kt : kt + 1, 0:1, hf : hf + 1],
                    )
                else:
                    e16 = E[:, ts(hf, 1024)].bitcast(i16)
                    nc.vector.tensor_scalar(
                        e16, ps[:], EXP_A2, EXP_B, ALU.mult, ALU.add
                    )

            def evac_bank(avp, bank, qout, eng):
                # evacuate one accumulator bank (4 q-tiles) to out rows
                # qout..qout+4
                ob = outp.tile([P, 4, P], bf16, tag=f"ob{qout}")
                src = avp[:, 4 * bank : 4 * bank + 4, :]
                if eng == "act":
                    nc.scalar.copy(ob[:], src)
                    nc.scalar.dma_start(out_d[:, qout : qout + 4, :], ob[:])
                else:
                    nc.vector.tensor_copy(ob[:], src)
                    nc.sync.dma_start(out_d[:, qout : qout + 4, :], ob[:])

            def av_part(kt, h, qlo, qhi, avp=None, qoff=0):
                # AV matmuls for ONE head: h1 early (its Vs closes during
                # kt), h0 late (its Vs closes early in kt+1)
                E = Etiles[kt][h]
                Vs = Vstiles[kt]
                if avp is None:
                    avp = avpA
                for qt in range(qlo, qhi):
                    qi = qt - qoff
                    # start=True zeroes the whole PSUM bank (4 q-tiles):
                    # only the first-emitted head (h1) on kt 0 carries it
                    nc.tensor.matmul(
                        avp[:, qi, ts(h, DH)],
                        lhsT=E[:, ts(qt, P)],
                        rhs=Vs[:, ts(h, DH)],
                        start=(kt == 0 and h == 1 and qi % 4 == 0),
                        stop=(kt == NKT - 1),
                        skip_group_check=True,
                    )

            def vproj_group(g):
                # V projection for k-tiles 4g..4g+3 in one 2-bank psum
                psv = spsum.tile([P, 1024], f32, tag="sco", name="psv")
                for j in range(4):
                    # j=0 / j=2 start=True zeroes banks 0 / 1 of the tile;
                    # j=1 / j=3 accumulate onto the already-zeroed bank
                    for dc in range(NDC):
                        nc.tensor.matmul(
                            psv[:, ts(j, P)],
                            lhsT=vt_s[:, dc, ts(4 * g + j, P)],
                            rhs=wv_s[:, dc, :],
                            start=(j % 2 == 0 and dc == 0),
                            stop=(dc == NDC - 1),
                            skip_group_check=True,
                        )
                # f32 so Pool's normalize_recip can read it directly
                Vf = vfpool.tile([P, 512], f32, tag="vf")
                if c["vf_eng"] == "act":
                    nc.scalar.copy(Vf[:], psv[:, 0:512])
                elif c["vf_eng"] == "dve":
                    nc.vector.tensor_copy(Vf[:], psv[:, 0:512])
                else:
                    nc.scalar.copy(Vf[:, 0:256], psv[:, 0:256])
                    nc.vector.tensor_copy(Vf[:, 256:512], psv[:, 256:512])
                return Vf

            def stats_early(kt):
                # h0 (ACT block) chain wrap-up for kt, run one iteration
                # late so the Pool add feeding it is already stale.  One
                # Pool op does Vs = Vf/D (and leaves 1/D behind).
                if kt < 0:
                    return
                Vf, voff = Vfinfo[kt]
                nc.gpsimd.normalize_recip(
                    Vstiles[kt][:, 0:DH],
                    Vf[:, voff : voff + DH],
                    Dtot[:, 2 * kt : 2 * kt + 1],
                )

            def stats_late(kt):
                # h1 (DVE block) chain: DVE 4x accum pass for D, then Pool
                # normalize_recip for Vs_h1.  The Pool add for h0's accum
                # halves is emitted last so it cannot delay Vs_h1.
                Vf, voff = Vfinfo[kt]
                scr = scrpool.tile([P, L], bf16, tag="scr")
                nc.vector.tensor_scalar(
                    scr[:],
                    Etiles[kt][1][:],
                    1.0,
                    0.0,
                    ALU.mult,
                    ALU.add,
                    accum_out=Dtot[:, 2 * kt + 1 : 2 * kt + 2],
                )
                Vs = vspool.tile([P, P], bf16, tag="vs")
                Vstiles[kt] = Vs
                nc.gpsimd.normalize_recip(
                    Vs[:, DH : 2 * DH],
                    Vf[:, voff + DH : voff + 2 * DH],
                    Dtot[:, 2 * kt + 1 : 2 * kt + 2],
                )
                if kt == 0:
                    # kt0's h0 accum: two quarter slices + one half slice
                    nc.gpsimd.tensor_add(Dq2[:, 0:1], Dq4[:, 0:1], Dq4[:, 1:2])
                    nc.gpsimd.tensor_add(
                        Dtot[:, 0:1], Dq2[:, 0:1], Dsum2[:, 0, 0, 1:2]
                    )
                else:
                    nc.gpsimd.tensor_add(
                        Dtot[:, 2 * kt : 2 * kt + 1],
                        Dsum2[:, kt, 0, 0:1],
                        Dsum2[:, kt, 0, 1:2],
                    )

            def alloc_E(kt):
                E0 = epool.tile([P, L], bf16, tag="E", name=f"E{kt}h0")
                E1 = epool.tile([P, L], bf16, tag="E", name=f"E{kt}h1")
                Etiles[kt] = (E0, E1)

            # warmup: projection spans gated only on their own input
            # chunks; kt0 scores at quarter granularity so the first exps
            # start right after qp0 + the 128-col kp_mini
            qk_proj_span(wq_s, qt_s, QT8, 0, 512, "act")
            qk_proj_span(wk_s, kt_s, KT8, 0, 512, "act")

            for kt in range(NKT):
                alloc_E(kt)
                E0, E1 = Etiles[kt]
                if kt == 0:
                    # first half at quarter granularity (starts on qp0
                    # alone); second half as normal [1024] halves
                    scores_quarter_kt0(0, 0, E0)
                    scores_quarter_kt0(1, 0, E1)
                    qk_proj_span(wq_s, qt_s, QT8, 512, 512, c["warm_qp1"])
                    scores_quarter_kt0(0, 1, E0)
                    scores_quarter_kt0(1, 1, E1)
                    qk_proj_span(wq_s, qt_s, QT8, 1024, 512, c["warm_qp23"])
                    qk_proj_span(wq_s, qt_s, QT8, 1536, 512, c["warm_qp23"])
                    scores_half(0, 0, 1, E0)
                    scores_half(0, 1, 1, E1)
                    Vf0 = vproj_group(0)
                    for j in range(4):
                        Vfinfo[j] = (Vf0, j * P)
                    stats_late(0)
                else:
                    scores_half(kt, 0, 0, E0)
                    scores_half(kt, 1, 0, E1)
                    stats_early(kt - 1)
                    if c["av1_pos"] == "after_early":
                        av_part(kt - 1, 1, 0, 8)
                    scores_half(kt, 0, 1, E0)
                    scores_half(kt, 1, 1, E1)
                    if c["av1_pos"] == "after_f1":
                        av_part(kt - 1, 1, 0, 8)
                    if c["av0_pos"] == "before_kproj":
                        av_part(kt - 1, 0, 0, 8)
                    if kt <= 3:
                        qk_proj_span(wk_s, kt_s, KT8, 512 * kt, 512, c["kproj_eng"])
                    g, r = divmod(kt + 1, 4)
                    if r == 0 and g < 4:
                        Vf = vproj_group(g)
                        for j in range(4):
                            Vfinfo[4 * g + j] = (Vf, j * P)
                    if c["av0_pos"] == "after_vproj":
                        av_part(kt - 1, 0, 0, 8)
                    stats_late(kt)

            # close kt=15's chains and the in-loop (q-tiles 0-7) AV
            stats_early(NKT - 1)
            av_part(NKT - 1, 1, 0, 8)
            av_part(NKT - 1, 0, 0, 8)

            # tail: replay AV for q-tiles 8-15 into a fresh accumulator
            # carved from the (now idle) score-psum pool, so the replay
            # runs concurrently with the qt0-7 evacs instead of waiting
            # for them.  kt order puts 15 last (its Vs closes latest).
            avpB = spsum.tile([P, 8, P], f32, tag="sco", name="avpB")
            kts = list(range(NKT - 1)) + [NKT - 1]
            evac_bank(avpA, 0, 0, "act")
            evac_bank(avpA, 1, 4, "dve")
            for kt in kts:
                av_part(kt, 1, 8, 12, avp=avpB, qoff=8)
                av_part(kt, 0, 8, 12, avp=avpB, qoff=8)
            evac_bank(avpB, 0, 8, "act")
            for kt in kts:
                av_part(kt, 1, 12, 16, avp=avpB, qoff=8)
                av_part(kt, 0, 12, 16, avp=avpB, qoff=8)
            evac_bank(avpB, 1, 12, "dve")

    nc.compile()
    return nc


def _get_program():
    if "nc" not in _CACHE:
        _CACHE["nc"] = _build_program()
    return _CACHE["nc"]


def kernel(keys, queries, values, WQ, WK, WV):
    import ml_dtypes

    from concourse import bass_utils

    bf = ml_dtypes.bfloat16
    keys = np.asarray(keys)
    queries = np.asarray(queries)
    values = np.asarray(values)
    WQ = np.asarray(WQ)
    WK = np.asarray(WK)
    WV = np.asarray(WV)

    nc = _get_program()

    in_maps = []
    for cc in range(N_CORES):
        b = cc // 4
        h0 = 2 * (cc % 4)
        h1 = h0 + 1
        in_maps.append(
            {
                "qt": np.ascontiguousarray(queries[b].T).astype(bf),
                "kt": np.ascontiguousarray(keys[b].T).astype(bf),
                "vt": np.ascontiguousarray(values[b].T).astype(bf),
                "wq": np.concatenate([WQ[h0], WQ[h1]], axis=1).astype(bf),
                "wk": np.concatenate([WK[h0], WK[h1]], axis=1).astype(bf),
                "wv": np.concatenate([WV[h0], WV[h1]], axis=1).astype(bf),
            }
        )

    res = bass_utils.run_bass_kernel_spmd(nc, in_maps, core_ids=list(range(N_CORES)))

    out = np.empty((B, L, H * DH), dtype=np.float32)
    for cc in range(N_CORES):
        b = cc // 4
        h0 = 2 * (cc % 4)
        ot = np.asarray(res.results[cc]["out"], dtype=np.float32)  # [128,16,128]
        out[b, :, h0 * DH : (h0 + 2) * DH] = ot.transpose(1, 0, 2).reshape(L, P)
    return out


# revision 65
# speedup vs baseline: 1.0049x; 1.0049x over previous
"""MultiHeadAttention Trainium2 kernel (8 NeuronCores, SPMD), v2.

Problem: B=2, L=2048, DK=DV=512, H=8, dh=64.
  Q = q @ WQ[h]; K = k @ WK[h]; V = v @ WV[h]       (per head)
  y = Q K^T / sqrt(L); z = softmax(y, axis=QUERY); out = z @ V
  concat heads on feature dim.

Sharding: 16 (b,h) pairs over 8 cores -> 2 heads (same batch) per core.

v2 vs v1 (81.3us -> 74.6us).  Engine busy went ACT 62.2->49.4,
DVE 57.8->49.9, PE 51.8->42.0; the wall is warmup-DMA lead + the
saturated, balanced exp/evac loop + the AV replay tail.
  * Scores matmuls run in fp8e4m3 with perf_mode=DoubleRow at 0.5
    cycles/row.  The 64-deep head contraction is packed to the required
    2-per-partition pair layout with stride-0 broadcast APs (each
    feature streamed twice -> computes 2*y; the 1/2 folds into the exp
    scale).  Q/K projections evacuate straight to fp8 (~+0.7% rel err,
    budget-checked).
  * Score PSUM tiles are [128, 1024] (2 banks); exp runs as halves,
    amortizing ACT's fixed access-latency + accum-read costs.  THREE
    score slots rotate so neither exp engine ever waits on a PSUM
    refill round-trip (2 slots measured +~0.7us/kt of exposed latency).
  * h0 exp exact on ACT (fused accum D); h1 on DVE Schraudolph
    (bf16-bits, whole blocks so the bias cancels in E/D) with a
    4x-mode copy-accum D pass.  Per-head stat chains are emitted so no
    engine's in-order queue fences on another engine's unfinished work
    (h0's D chain closes one iteration late when its deps are stale).
  * The 8 PSUM banks can't hold 3 score slots AND a 16-q-tile AV
    accumulator: the loop accumulates q-tiles 0-7 in 2 banks; q-tiles
    8-15 replay over the all-live E tiles in a PE-only tail into a
    fresh accumulator carved from the then-idle score pool, with the
    evacs overlapped bank-by-bank.
  * V projection batched 4 k-tiles per PSUM tile, one f32 evac; Pool's
    normalize_recip does Vs = Vf*(1/D) in one op (no DVE reciprocals).
  * kt0 scores at 512-wide quarters so the first exps start as soon as
    the first q/k projection spans land; dummy PE matmuls bridge the
    input-DMA wait so the HAM p-state ramp completes before real work.
"""

import math

import numpy as np

B = 2
L = 2048
DK = 512
H = 8
DH = 64
P = 128
NKT = L // P  # 16 k-tiles
NDC = DK // P  # 4 d-chunks
N_CORES = 8

SCALE = 1.0 / math.sqrt(float(L))
# Schraudolph in bf16-bits domain: round(raw*EXP_A2 + EXP_B) as int16,
# bitcast bf16 ~= exp(raw*SCALE).  Raw fp8 scores are 2*y so EXP_A2
# carries a 1/2.
EXP_A2 = 128.0 * math.log2(math.e) * SCALE * 0.5
EXP_B = 16256.0 - 12.0
SCALE2 = SCALE * 0.5  # ACT exp scale on the doubled raw scores

# schedule knobs (swept via TimelineSim)
CFG = dict(
    w_queue="scalar",  # weight DMA issue queue: 'sync' | 'scalar'
    dma_variant=0,  # 0: baseline order, 1: wq,wk first
    warm_qp1="dve",  # engine for warmup q-quarter-1 evac
    warm_qp23="dve",  # engine for kt0 q-quarter-2/3 evacs
    kproj_eng="act",  # engine for kproj quarter evacs (kts 1-3)
    av0_pos="after_vproj",  # 'before_kproj' | 'after_vproj'
    av1_pos="after_early",  # 'after_early' | 'after_f1'
    vf_eng="act",  # 'act' | 'dve' | 'split'
    scr_bufs=2,
    vf_bufs=3,
    prewarm_mms=36,
    vproj_pos="mid",  # 'cur' (at kt=4g-1, late) | 'mid' (at kt=4g, mid-kt)
)

_CACHE = {}


def _build_program(cfg=None):
    import concourse.bass as bass
    import concourse.tile as tile
    from concourse import bacc, mybir
    from concourse.bass import ts

    c = dict(CFG)
    if cfg:
        c.update(cfg)

    f32 = mybir.dt.float32
    bf16 = mybir.dt.bfloat16
    fp8 = mybir.dt.float8e4
    i16 = mybir.dt.int16
    AF = mybir.ActivationFunctionType
    ALU = mybir.AluOpType
    DR = mybir.MatmulPerfMode.DoubleRow

    nc = bacc.Bacc("TRN2", target_bir_lowering=False, debug=False)

    qt_d = nc.dram_tensor("qt", [DK, L], bf16, kind="ExternalInput")
    kt_d = nc.dram_tensor("kt", [DK, L], bf16, kind="ExternalInput")
    vt_d = nc.dram_tensor("vt", [DK, L], bf16, kind="ExternalInput")
    wq_d = nc.dram_tensor("wq", [DK, P], bf16, kind="ExternalInput")
    wk_d = nc.dram_tensor("wk", [DK, P], bf16, kind="ExternalInput")
    wv_d = nc.dram_tensor("wv", [DK, P], bf16, kind="ExternalInput")
    out_d = nc.dram_tensor("out", [P, NKT, P], bf16, kind="ExternalOutput")

    with tile.TileContext(nc) as tc:
        with (
            tc.tile_pool(name="consts", bufs=1) as consts,
            tc.tile_pool(name="xin", bufs=1) as xin,
            tc.tile_pool(name="proj", bufs=1) as proj,
            tc.tile_pool(name="epool", bufs=2 * NKT) as epool,
            tc.tile_pool(name="vfpool", bufs=c["vf_bufs"]) as vfpool,
            tc.tile_pool(name="scrpool", bufs=c["scr_bufs"]) as scrpool,
            tc.tile_pool(name="vspool", bufs=NKT) as vspool,
            tc.tile_pool(name="stats", bufs=1) as stats,
            tc.tile_pool(name="outp", bufs=1) as outp,
            tc.tile_pool(name="spsum", bufs=3, space="PSUM") as spsum,
            tc.tile_pool(name="avpsum", bufs=1, space="PSUM") as avpsum,
        ):
            wq_s = consts.tile([P, NDC, P], bf16)
            wk_s = consts.tile([P, NDC, P], bf16)
            wv_s = consts.tile([P, NDC, P], bf16)
            qt_s = xin.tile([P, NDC, L], bf16)
            kt_s = xin.tile([P, NDC, L], bf16)
            vt_s = xin.tile([P, NDC, L], bf16)
            qt_r = qt_d.rearrange("(o p) l -> p o l", p=P)
            kt_r = kt_d.rearrange("(o p) l -> p o l", p=P)
            vt_r = vt_d.rearrange("(o p) l -> p o l", p=P)

            def load_chunk(sb, rr, ch):
                nc.sync.dma_start(sb[:, :, ts(ch, 512)], rr[:, :, ts(ch, 512)])

            def load_w(sb, rr):
                r = rr.rearrange("(o p) e -> p o e", p=P)
                if c["w_queue"] == "scalar":
                    nc.scalar.dma_start(sb[:], r)
                else:
                    nc.sync.dma_start(sb[:], r)

            # PE p-state: dummy matmuls from t~0 keep the PE busy through
            # the input-DMA wait so the 3.4us HAM ramp completes (and
            # pe_busy_start never resets) before the first projection
            zt = consts.tile([P, P], bf16)
            nc.gpsimd.memset(zt[:], 0.0)
            pwp = spsum.tile([P, 1024], f32, tag="sco", name="prewarm")
            for _ in range(c["prewarm_mms"]):
                nc.tensor.matmul(
                    pwp[:, 0:P], lhsT=zt[:], rhs=zt[:], start=True, stop=True
                )

            if c["dma_variant"] == 0:
                # k0a is a tiny 128-col K slice so the kp_mini projection
                # (and with it kt0's first score quarters) starts ~4us
                # before the full k-chunk lands
                load_w(wq_s, wq_d)
                load_chunk(qt_s, qt_r, 0)
                load_w(wk_s, wk_d)
                load_chunk(kt_s, kt_r, 0)
                load_chunk(qt_s, qt_r, 1)
                load_chunk(qt_s, qt_r, 2)
                load_chunk(qt_s, qt_r, 3)
                load_w(wv_s, wv_d)
                load_chunk(vt_s, vt_r, 0)
                load_chunk(kt_s, kt_r, 1)
                load_chunk(kt_s, kt_r, 2)
                load_chunk(vt_s, vt_r, 1)
                load_chunk(kt_s, kt_r, 3)
                load_chunk(vt_s, vt_r, 2)
                load_chunk(vt_s, vt_r, 3)
            else:
                load_w(wq_s, wq_d)
                load_chunk(qt_s, qt_r, 0)
                load_w(wk_s, wk_d)
                load_chunk(kt_s, kt_r, 0)
                load_chunk(qt_s, qt_r, 1)
                load_chunk(qt_s, qt_r, 2)
                load_chunk(qt_s, qt_r, 3)
                load_w(wv_s, wv_d)
                load_chunk(vt_s, vt_r, 0)
                load_chunk(kt_s, kt_r, 1)
                load_chunk(kt_s, kt_r, 2)
                load_chunk(vt_s, vt_r, 1)
                load_chunk(kt_s, kt_r, 3)
                load_chunk(vt_s, vt_r, 2)
                load_chunk(vt_s, vt_r, 3)

            # fp8 projected Q/K: head h features on partitions h*64..h*64+64
            QT8 = proj.tile([P, L], fp8)
            KT8 = proj.tile([P, L], fp8)

            Dsum2 = stats.tile([P, NKT, 2, 2], f32)  # ACT accum halves
            Dtot = stats.tile([P, NKT * 2], f32)
            Dq4 = stats.tile([P, 4], f32)  # kt0 quarter accums (h0)
            Dq2 = stats.tile([P, 2], f32)

            # AV accumulator: 2 banks hold q-tiles 0-7 during the kt loop;
            # q-tiles 8-15 replay over the all-live E tiles in a PE tail
            # that reuses the same banks after the first-half evac
            avpA = avpsum.tile([P, 8, P], f32, tag="ava")

            def qk_proj_span(W, X, OUT8, lo, width, eng):
                # project cols [lo, lo+width): gated only on the input
                # chunks covering that span
                ps = spsum.tile([P, 1024], f32, tag="sco", name="qkproj")
                for dc in range(NDC):
                    nc.tensor.matmul(
                        ps[:, 0:width],
                        lhsT=W[:, dc, :],
                        rhs=X[:, dc, lo : lo + width],
                        start=(dc == 0),
                        stop=(dc == NDC - 1),
                    )
                if eng == "act":
                    nc.scalar.copy(OUT8[:, lo : lo + width], ps[:, 0:width])
                else:
                    nc.vector.tensor_copy(OUT8[:, lo : lo + width], ps[:, 0:width])

            Etiles = {}
            Vstiles = {}
            Vfinfo = {}

            def scores_quarter_kt0(h, qc, E):
                # kt0 runs at 512-wide quarters so the first exps start as
                # soon as each q-projection span lands
                hp = h * DH
                ps = spsum.tile([P, 1024], f32, tag="sco", name="sco0")
                lhs = KT8[hp : hp + DH, 0:P].unsqueeze(1).broadcast_to([DH, 2, P])
                rhs = (
                    QT8[hp : hp + DH, ts(qc, 512)]
                    .unsqueeze(1)
                    .broadcast_to([DH, 2, 512])
                )
                nc.tensor.matmul(
                    ps[:, 0:512], lhsT=lhs, rhs=rhs, start=True, stop=True,
                    perf_mode=DR,
                )
                if h == 0:
                    nc.scalar.activation(
                        E[:, ts(qc, 512)],
                        ps[:, 0:512],
                        AF.Exp,
                        scale=SCALE2,
                        accum_out=Dq4[:, qc : qc + 1],
                    )
                else:
                    e16 = E[:, ts(qc, 512)].bitcast(i16)
                    nc.vector.tensor_scalar(
                        e16, ps[:, 0:512], EXP_A2, EXP_B, ALU.mult, ALU.add
                    )

            def scores_half(kt, h, hf, E):
                # one [128, 1024] half: 2 DoubleRow fp8 matmuls + one exp
                hp = h * DH
                ps = spsum.tile([P, 1024], f32, tag="sco", name="sco")
                for sub in range(2):
                    qc = hf * 2 + sub
                    lhs = (
                        KT8[hp : hp + DH, ts(kt, P)]
                        .unsqueeze(1)
                        .broadcast_to([DH, 2, P])
                    )
                    rhs = (
                        QT8[hp : hp + DH, ts(qc, 512)]
                        .unsqueeze(1)
                        .broadcast_to([DH, 2, 512])
                    )
                    nc.tensor.matmul(
                        ps[:, ts(sub, 512)],
                        lhsT=lhs,
                        rhs=rhs,
                        start=True,
                        stop=True,
                        perf_mode=DR,
                    )
                if h == 0:
                    nc.scalar.activation(
                        E[:, ts(hf, 1024)],
                        ps[:],
                        AF.Exp,
                        scale=SCALE2,
                        accum_out=Dsum2[:, kt : kt + 1, 0:1, hf : hf + 1],
                    )
                else:
                    e16 = E[:, ts(hf, 1024)].bitcast(i16)
                    nc.vector.tensor_scalar(
                        e16, ps[:], EXP_A2, EXP_B, ALU.mult, ALU.add
                    )

            def evac_bank(avp, bank, qout, eng):
                # evacuate one accumulator bank (4 q-tiles) to out rows
                # qout..qout+4
                ob = outp.tile([P, 4, P], bf16, tag=f"ob{qout}")
                src = avp[:, 4 * bank : 4 * bank + 4, :]
                if eng == "act":
                    nc.scalar.copy(ob[:], src)
                    nc.scalar.dma_start(out_d[:, qout : qout + 4, :], ob[:])
                else:
                    nc.vector.tensor_copy(ob[:], src)
                    nc.sync.dma_start(out_d[:, qout : qout + 4, :], ob[:])

            def av_part(kt, h, qlo, qhi, avp=None, qoff=0):
                # AV matmuls for ONE head: h1 early (its Vs closes during
                # kt), h0 late (its Vs closes early in kt+1)
                E = Etiles[kt][h]
                Vs = Vstiles[kt]
                if avp is None:
                    avp = avpA
                for qt in range(qlo, qhi):
                    qi = qt - qoff
                    # start=True zeroes the whole PSUM bank (4 q-tiles):
                    # only the first-emitted head (h1) on kt 0 carries it
                    nc.tensor.matmul(
                        avp[:, qi, ts(h, DH)],
                        lhsT=E[:, ts(qt, P)],
                        rhs=Vs[:, ts(h, DH)],
                        start=(kt == 0 and h == 1 and qi % 4 == 0),
                        stop=(kt == NKT - 1),
                        skip_group_check=True,
                    )

            def vproj_group(g):
                # V projection for k-tiles 4g..4g+3 in one 2-bank psum
                psv = spsum.tile([P, 1024], f32, tag="sco", name="psv")
                for j in range(4):
                    # j=0 / j=2 start=True zeroes banks 0 / 1 of the tile;
                    # j=1 / j=3 accumulate onto the already-zeroed bank
                    for dc in range(NDC):
                        nc.tensor.matmul(
                            psv[:, ts(j, P)],
                            lhsT=vt_s[:, dc, ts(4 * g + j, P)],
                            rhs=wv_s[:, dc, :],
                            start=(j % 2 == 0 and dc == 0),
                            stop=(dc == NDC - 1),
                            skip_group_check=True,
                        )
                # f32 so Pool's normalize_recip can read it directly
                Vf = vfpool.tile([P, 512], f32, tag="vf")
                if c["vf_eng"] == "act":
                    nc.scalar.copy(Vf[:], psv[:, 0:512])
                elif c["vf_eng"] == "dve":
                    nc.vector.tensor_copy(Vf[:], psv[:, 0:512])
                else:
                    nc.scalar.copy(Vf[:, 0:256], psv[:, 0:256])
                    nc.vector.tensor_copy(Vf[:, 256:512], psv[:, 256:512])
                return Vf

            def stats_early(kt):
                # h0 (ACT block) chain wrap-up for kt, run one iteration
                # late so the Pool add feeding it is already stale.  One
                # Pool op does Vs = Vf/D (and leaves 1/D behind).
                if kt < 0:
                    return
                Vf, voff = Vfinfo[kt]
                nc.gpsimd.normalize_recip(
                    Vstiles[kt][:, 0:DH],
                    Vf[:, voff : voff + DH],
                    Dtot[:, 2 * kt : 2 * kt + 1],
                )

            def stats_late(kt):
                # h1 (DVE block) chain: DVE 4x accum pass for D, then Pool
                # normalize_recip for Vs_h1.  The Pool add for h0's accum
                # halves is emitted last so it cannot delay Vs_h1.
                Vf, voff = Vfinfo[kt]
                scr = scrpool.tile([P, L], bf16, tag="scr")
                nc.vector.tensor_scalar(
                    scr[:],
                    Etiles[kt][1][:],
                    1.0,
                    0.0,
                    ALU.mult,
                    ALU.add,
                    accum_out=Dtot[:, 2 * kt + 1 : 2 * kt + 2],
                )
                Vs = vspool.tile([P, P], bf16, tag="vs")
                Vstiles[kt] = Vs
                nc.gpsimd.normalize_recip(
                    Vs[:, DH : 2 * DH],
                    Vf[:, voff + DH : voff + 2 * DH],
                    Dtot[:, 2 * kt + 1 : 2 * kt + 2],
                )
                if kt == 0:
                    # kt0's h0 accum: two quarter slices + one half slice
                    nc.gpsimd.tensor_add(Dq2[:, 0:1], Dq4[:, 0:1], Dq4[:, 1:2])
                    nc.gpsimd.tensor_add(
                        Dtot[:, 0:1], Dq2[:, 0:1], Dsum2[:, 0, 0, 1:2]
                    )
                else:
                    nc.gpsimd.tensor_add(
                        Dtot[:, 2 * kt : 2 * kt + 1],
                        Dsum2[:, kt, 0, 0:1],
                        Dsum2[:, kt, 0, 1:2],
                    )

            def alloc_E(kt):
                E0 = epool.tile([P, L], bf16, tag="E", name=f"E{kt}h0")
                E1 = epool.tile([P, L], bf16, tag="E", name=f"E{kt}h1")
                Etiles[kt] = (E0, E1)

            # warmup: projection spans gated only on their own input
            # chunks; kt0 scores at quarter granularity so the first exps
            # start right after qp0 + the 128-col kp_mini
            qk_proj_span(wq_s, qt_s, QT8, 0, 512, "act")
            qk_proj_span(wk_s, kt_s, KT8, 0, 512, "act")

            for kt in range(NKT):
                alloc_E(kt)
                E0, E1 = Etiles[kt]
                if kt == 0:
                    # first half at quarter granularity (starts on qp0
                    # alone); second half as normal [1024] halves
                    scores_quarter_kt0(0, 0, E0)
                    scores_quarter_kt0(1, 0, E1)
                    qk_proj_span(wq_s, qt_s, QT8, 512, 512, c["warm_qp1"])
                    scores_quarter_kt0(0, 1, E0)
                    scores_quarter_kt0(1, 1, E1)
                    qk_proj_span(wq_s, qt_s, QT8, 1024, 512, c["warm_qp23"])
                    qk_proj_span(wq_s, qt_s, QT8, 1536, 512, c["warm_qp23"])
                    scores_half(0, 0, 1, E0)
                    scores_half(0, 1, 1, E1)
                    Vf0 = vproj_group(0)
                    for j in range(4):
                        Vfinfo[j] = (Vf0, j * P)
                    stats_late(0)
                else:
                    scores_half(kt, 0, 0, E0)
                    scores_half(kt, 1, 0, E1)
                    stats_early(kt - 1)
                    if c["av1_pos"] == "after_early":
                        av_part(kt - 1, 1, 0, 8)
                    if c["vproj_pos"] == "mid" and kt % 4 == 0:
                        Vf = vproj_group(kt // 4)
                        for j in range(4):
                            Vfinfo[kt + j] = (Vf, j * P)
                    scores_half(kt, 0, 1, E0)
                    scores_half(kt, 1, 1, E1)
                    if c["av1_pos"] == "after_f1":
                        av_part(kt - 1, 1, 0, 8)
                    if c["av0_pos"] == "before_kproj":
                        av_part(kt - 1, 0, 0, 8)
                    if kt <= 3:
                        qk_proj_span(wk_s, kt_s, KT8, 512 * kt, 512, c["kproj_eng"])
                    if c["vproj_pos"] == "cur":
                        g, r = divmod(kt + 1, 4)
                        if r == 0 and g < 4:
                            Vf = vproj_group(g)
                            for j in range(4):
                                Vfinfo[4 * g + j] = (Vf, j * P)
                    if c["av0_pos"] == "after_vproj":
                        av_part(kt - 1, 0, 0, 8)
                    stats_late(kt)

            # close kt=15's chains and the in-loop (q-tiles 0-7) AV
            stats_early(NKT - 1)
            av_part(NKT - 1, 1, 0, 8)
            av_part(NKT - 1, 0, 0, 8)

            # tail: replay AV for q-tiles 8-15 into a fresh accumulator
            # carved from the (now idle) score-psum pool, so the replay
            # runs concurrently with the qt0-7 evacs instead of waiting
            # for them.  kt order puts 15 last (its Vs closes latest).
            avpB = spsum.tile([P, 8, P], f32, tag="sco", name="avpB")
            kts = list(range(NKT - 1)) + [NKT - 1]
            evac_bank(avpA, 0, 0, "act")
            evac_bank(avpA, 1, 4, "dve")
            for kt in kts:
                av_part(kt, 1, 8, 12, avp=avpB, qoff=8)
                av_part(kt, 0, 8, 12, avp=avpB, qoff=8)
            evac_bank(avpB, 0, 8, "act")
            for kt in kts:
                av_part(kt, 1, 12, 16, avp=avpB, qoff=8)
                av_part(kt, 0, 12, 16, avp=avpB, qoff=8)
            evac_bank(avpB, 1, 12, "dve")

    nc.compile()
    return nc


def _get_program():
    if "nc" not in _CACHE:
        _CACHE["nc"] = _build_program()
    return _CACHE["nc"]


def kernel(keys, queries, values, WQ, WK, WV):
    import ml_dtypes

    from concourse import bass_utils

    bf = ml_dtypes.bfloat16
    keys = np.asarray(keys)
    queries = np.asarray(queries)
    values = np.asarray(values)
    WQ = np.asarray(WQ)
    WK = np.asarray(WK)
    WV = np.asarray(WV)

    nc = _get_program()

    in_maps = []
    for cc in range(N_CORES):
        b = cc // 4
        h0 = 2 * (cc % 4)
        h1 = h0 + 1
        in_maps.append(
            {
                "qt": np.ascontiguousarray(queries[b].T).astype(bf),
                "kt": np.ascontiguousarray(keys[b].T).astype(bf),
                "vt": np.ascontiguousarray(values[b].T).astype(bf),
                "wq": np.concatenate([WQ[h0], WQ[h1]], axis=1).astype(bf),
                "wk": np.concatenate([WK[h0], WK[h1]], axis=1).astype(bf),
                "wv": np.concatenate([WV[h0], WV[h1]], axis=1).astype(bf),
            }
        )

    res = bass_utils.run_bass_kernel_spmd(nc, in_maps, core_ids=list(range(N_CORES)))

    out = np.empty((B, L, H * DH), dtype=np.float32)
    for cc in range(N_CORES):
        b = cc // 4
        h0 = 2 * (cc % 4)
        ot = np.asarray(res.results[cc]["out"], dtype=np.float32)  # [128,16,128]
        out[b, :, h0 * DH : (h0 + 2) * DH] = ot.transpose(1, 0, 2).reshape(L, P)
    return out


# revision 66
# speedup vs baseline: 1.0244x; 1.0194x over previous
"""MultiHeadAttention Trainium2 kernel (8 NeuronCores, SPMD), v2.

Problem: B=2, L=2048, DK=DV=512, H=8, dh=64.
  Q = q @ WQ[h]; K = k @ WK[h]; V = v @ WV[h]       (per head)
  y = Q K^T / sqrt(L); z = softmax(y, axis=QUERY); out = z @ V
  concat heads on feature dim.

Sharding: 16 (b,h) pairs over 8 cores -> 2 heads (same batch) per core.

v2 vs v1 (81.3us -> 74.6us).  Engine busy went ACT 62.2->49.4,
DVE 57.8->49.9, PE 51.8->42.0; the wall is warmup-DMA lead + the
saturated, balanced exp/evac loop + the AV replay tail.
  * Scores matmuls run in fp8e4m3 with perf_mode=DoubleRow at 0.5
    cycles/row.  The 64-deep head contraction is packed to the required
    2-per-partition pair layout with stride-0 broadcast APs (each
    feature streamed twice -> computes 2*y; the 1/2 folds into the exp
    scale).  Q/K projections evacuate straight to fp8 (~+0.7% rel err,
    budget-checked).
  * Score PSUM tiles are [128, 1024] (2 banks); exp runs as halves,
    amortizing ACT's fixed access-latency + accum-read costs.  THREE
    score slots rotate so neither exp engine ever waits on a PSUM
    refill round-trip (2 slots measured +~0.7us/kt of exposed latency).
  * h0 exp exact on ACT (fused accum D); h1 on DVE Schraudolph
    (bf16-bits, whole blocks so the bias cancels in E/D) with a
    4x-mode copy-accum D pass.  Per-head stat chains are emitted so no
    engine's in-order queue fences on another engine's unfinished work
    (h0's D chain closes one iteration late when its deps are stale).
  * The 8 PSUM banks can't hold 3 score slots AND a 16-q-tile AV
    accumulator: the loop accumulates q-tiles 0-7 in 2 banks; q-tiles
    8-15 replay over the all-live E tiles in a PE-only tail into a
    fresh accumulator carved from the then-idle score pool, with the
    evacs overlapped bank-by-bank.
  * V projection batched 4 k-tiles per PSUM tile, one f32 evac; Pool's
    normalize_recip does Vs = Vf*(1/D) in one op (no DVE reciprocals).
  * kt0 scores at 512-wide quarters so the first exps start as soon as
    the first q/k projection spans land; dummy PE matmuls bridge the
    input-DMA wait so the HAM p-state ramp completes before real work.
"""

import math

import numpy as np

B = 2
L = 2048
DK = 512
H = 8
DH = 64
P = 128
NKT = L // P  # 16 k-tiles
NDC = DK // P  # 4 d-chunks
N_CORES = 8

SCALE = 1.0 / math.sqrt(float(L))
# Schraudolph in bf16-bits domain: round(raw*EXP_A2 + EXP_B) as int16,
# bitcast bf16 ~= exp(raw*SCALE).  Raw fp8 scores are 2*y so EXP_A2
# carries a 1/2.
EXP_A2 = 128.0 * math.log2(math.e) * SCALE * 0.5
EXP_B = 16256.0 - 12.0
SCALE2 = SCALE * 0.5  # ACT exp scale on the doubled raw scores

# schedule knobs (swept via TimelineSim)
CFG = dict(
    w_queue="scalar",  # weight DMA issue queue: 'sync' | 'scalar'
    dma_variant=0,  # 0: baseline order, 1: wq,wk first
    warm_qp1="dve",  # engine for warmup q-quarter-1 evac
    warm_qp23="dve",  # engine for kt0 q-quarter-2/3 evacs
    kproj_eng="act",  # engine for kproj quarter evacs (kts 1-3)
    av0_pos="after_vproj",  # 'before_kproj' | 'after_vproj'
    av1_pos="after_early",  # 'after_early' | 'after_f1'
    vf_eng="act",  # 'act' | 'dve' | 'split'
    scr_bufs=2,
    vf_bufs=3,
    prewarm_mms=36,
    vproj_pos="mid",  # 'cur' (at kt=4g-1, late) | 'mid' (at kt=4g, mid-kt)
)

_CACHE = {}


def _build_program(cfg=None):
    import concourse.bass as bass
    import concourse.tile as tile
    from concourse import bacc, mybir
    from concourse.bass import ts

    c = dict(CFG)
    if cfg:
        c.update(cfg)

    f32 = mybir.dt.float32
    bf16 = mybir.dt.bfloat16
    fp8 = mybir.dt.float8e4
    i16 = mybir.dt.int16
    AF = mybir.ActivationFunctionType
    ALU = mybir.AluOpType
    DR = mybir.MatmulPerfMode.DoubleRow

    nc = bacc.Bacc("TRN2", target_bir_lowering=False, debug=False)

    qt_d = nc.dram_tensor("qt", [DK, L], bf16, kind="ExternalInput")
    kt_d = nc.dram_tensor("kt", [DK, L], bf16, kind="ExternalInput")
    vt_d = nc.dram_tensor("vt", [DK, L], bf16, kind="ExternalInput")
    wq_d = nc.dram_tensor("wq", [DK, P], bf16, kind="ExternalInput")
    wk_d = nc.dram_tensor("wk", [DK, P], bf16, kind="ExternalInput")
    wv_d = nc.dram_tensor("wv", [DK, P], bf16, kind="ExternalInput")
    out_d = nc.dram_tensor("out", [P, NKT, P], bf16, kind="ExternalOutput")

    with tile.TileContext(nc) as tc:
        with (
            tc.tile_pool(name="consts", bufs=1) as consts,
            tc.tile_pool(name="xin", bufs=1) as xin,
            tc.tile_pool(name="proj", bufs=1) as proj,
            tc.tile_pool(name="epool", bufs=2 * NKT) as epool,
            tc.tile_pool(name="vfpool", bufs=c["vf_bufs"]) as vfpool,
            tc.tile_pool(name="scrpool", bufs=c["scr_bufs"]) as scrpool,
            tc.tile_pool(name="vspool", bufs=NKT) as vspool,
            tc.tile_pool(name="stats", bufs=1) as stats,
            tc.tile_pool(name="outp", bufs=1) as outp,
            tc.tile_pool(name="spsum", bufs=3, space="PSUM") as spsum,
            tc.tile_pool(name="avpsum", bufs=1, space="PSUM") as avpsum,
        ):
            wq_s = consts.tile([P, NDC, P], bf16)
            wk_s = consts.tile([P, NDC, P], bf16)
            wv_s = consts.tile([P, NDC, P], bf16)
            qt_s = xin.tile([P, NDC, L], bf16)
            kt_s = xin.tile([P, NDC, L], bf16)
            vt_s = xin.tile([P, NDC, L], bf16)
            qt_r = qt_d.rearrange("(o p) l -> p o l", p=P)
            kt_r = kt_d.rearrange("(o p) l -> p o l", p=P)
            vt_r = vt_d.rearrange("(o p) l -> p o l", p=P)

            def load_chunk(sb, rr, ch):
                nc.sync.dma_start(sb[:, :, ts(ch, 512)], rr[:, :, ts(ch, 512)])

            def load_w(sb, rr):
                r = rr.rearrange("(o p) e -> p o e", p=P)
                if c["w_queue"] == "scalar":
                    nc.scalar.dma_start(sb[:], r)
                else:
                    nc.sync.dma_start(sb[:], r)

            # PE p-state: dummy matmuls from t~0 keep the PE busy through
            # the input-DMA wait so the 3.4us HAM ramp completes (and
            # pe_busy_start never resets) before the first projection
            zt = consts.tile([P, P], bf16)
            nc.gpsimd.memset(zt[:], 0.0)
            pwp = spsum.tile([P, 1024], f32, tag="sco", name="prewarm")
            for _ in range(c["prewarm_mms"]):
                nc.tensor.matmul(
                    pwp[:, 0:P], lhsT=zt[:], rhs=zt[:], start=True, stop=True
                )

            if c["dma_variant"] == 0:
                # k0a is a tiny 128-col K slice so the kp_mini projection
                # (and with it kt0's first score quarters) starts ~4us
                # before the full k-chunk lands
                load_w(wq_s, wq_d)
                load_chunk(qt_s, qt_r, 0)
                load_w(wk_s, wk_d)
                load_chunk(kt_s, kt_r, 0)
                load_chunk(qt_s, qt_r, 1)
                load_chunk(qt_s, qt_r, 2)
                load_chunk(qt_s, qt_r, 3)
                load_w(wv_s, wv_d)
                load_chunk(vt_s, vt_r, 0)
                load_chunk(kt_s, kt_r, 1)
                load_chunk(kt_s, kt_r, 2)
                load_chunk(vt_s, vt_r, 1)
                load_chunk(kt_s, kt_r, 3)
                load_chunk(vt_s, vt_r, 2)
                load_chunk(vt_s, vt_r, 3)
            else:
                load_w(wq_s, wq_d)
                load_chunk(qt_s, qt_r, 0)
                load_w(wk_s, wk_d)
                load_chunk(kt_s, kt_r, 0)
                load_chunk(qt_s, qt_r, 1)
                load_chunk(qt_s, qt_r, 2)
                load_chunk(qt_s, qt_r, 3)
                load_w(wv_s, wv_d)
                load_chunk(vt_s, vt_r, 0)
                load_chunk(kt_s, kt_r, 1)
                load_chunk(kt_s, kt_r, 2)
                load_chunk(vt_s, vt_r, 1)
                load_chunk(kt_s, kt_r, 3)
                load_chunk(vt_s, vt_r, 2)
                load_chunk(vt_s, vt_r, 3)

            # fp8 projected Q/K: head h features on partitions h*64..h*64+64
            QT8 = proj.tile([P, L], fp8)
            KT8 = proj.tile([P, L], fp8)

            Dsum2 = stats.tile([P, NKT, 2, 2], f32)  # ACT accum halves
            Dtot = stats.tile([P, NKT * 2], f32)
            Dq4 = stats.tile([P, 4], f32)  # kt0 quarter accums (h0)
            Dq2 = stats.tile([P, 2], f32)

            # AV accumulator: 2 banks hold q-tiles 0-7 during the kt loop;
            # q-tiles 8-15 replay over the all-live E tiles in a PE tail
            # that reuses the same banks after the first-half evac
            avpA = avpsum.tile([P, 8, P], f32, tag="ava")

            def qk_proj_span(W, X, OUT8, lo, width, eng):
                # project cols [lo, lo+width): gated only on the input
                # chunks covering that span
                ps = spsum.tile([P, 1024], f32, tag="sco", name="qkproj")
                for dc in range(NDC):
                    nc.tensor.matmul(
                        ps[:, 0:width],
                        lhsT=W[:, dc, :],
                        rhs=X[:, dc, lo : lo + width],
                        start=(dc == 0),
                        stop=(dc == NDC - 1),
                    )
                if eng == "act":
                    nc.scalar.copy(OUT8[:, lo : lo + width], ps[:, 0:width])
                else:
                    nc.vector.tensor_copy(OUT8[:, lo : lo + width], ps[:, 0:width])

            Etiles = {}
            Vstiles = {}
            Vfinfo = {}

            def scores_quarter_kt0(h, qc, E):
                # kt0 runs at 512-wide quarters so the first exps start as
                # soon as each q-projection span lands
                hp = h * DH
                ps = spsum.tile([P, 1024], f32, tag="sco", name="sco0")
                lhs = KT8[hp : hp + DH, 0:P].unsqueeze(1).broadcast_to([DH, 2, P])
                rhs = (
                    QT8[hp : hp + DH, ts(qc, 512)]
                    .unsqueeze(1)
                    .broadcast_to([DH, 2, 512])
                )
                nc.tensor.matmul(
                    ps[:, 0:512], lhsT=lhs, rhs=rhs, start=True, stop=True,
                    perf_mode=DR,
                )
                if h == 0:
                    nc.scalar.activation(
                        E[:, ts(qc, 512)],
                        ps[:, 0:512],
                        AF.Exp,
                        scale=SCALE2,
                        accum_out=Dq4[:, qc : qc + 1],
                    )
                else:
                    e16 = E[:, ts(qc, 512)].bitcast(i16)
                    nc.vector.tensor_scalar(
                        e16, ps[:, 0:512], EXP_A2, EXP_B, ALU.mult, ALU.add
                    )

            def scores_half(kt, h, hf, E):
                # one [128, 1024] half: 2 DoubleRow fp8 matmuls + one exp
                hp = h * DH
                ps = spsum.tile([P, 1024], f32, tag="sco", name="sco")
                for sub in range(2):
                    qc = hf * 2 + sub
                    lhs = (
                        KT8[hp : hp + DH, ts(kt, P)]
                        .unsqueeze(1)
                        .broadcast_to([DH, 2, P])
                    )
                    rhs = (
                        QT8[hp : hp + DH, ts(qc, 512)]
                        .unsqueeze(1)
                        .broadcast_to([DH, 2, 512])
                    )
                    nc.tensor.matmul(
                        ps[:, ts(sub, 512)],
                        lhsT=lhs,
                        rhs=rhs,
                        start=True,
                        stop=True,
                        perf_mode=DR,
                    )
                if h == 0:
                    nc.scalar.activation(
                        E[:, ts(hf, 1024)],
                        ps[:],
                        AF.Exp,
                        scale=SCALE2,
                        accum_out=Dsum2[:, kt : kt + 1, 0:1, hf : hf + 1],
                    )
                else:
                    e16 = E[:, ts(hf, 1024)].bitcast(i16)
                    nc.vector.tensor_scalar(
                        e16, ps[:], EXP_A2, EXP_B, ALU.mult, ALU.add
                    )

            def evac_bank(avp, bank, qout, eng):
                # evacuate one accumulator bank (4 q-tiles) to out rows
                # qout..qout+4
                ob = outp.tile([P, 4, P], bf16, tag=f"ob{qout}")
                src = avp[:, 4 * bank : 4 * bank + 4, :]
                if eng == "act":
                    nc.scalar.copy(ob[:], src)
                    nc.scalar.dma_start(out_d[:, qout : qout + 4, :], ob[:])
                else:
                    nc.vector.tensor_copy(ob[:], src)
                    nc.sync.dma_start(out_d[:, qout : qout + 4, :], ob[:])

            def av_part(kt, h, qlo, qhi, avp=None, qoff=0):
                # AV matmuls for ONE head: h1 early (its Vs closes during
                # kt), h0 late (its Vs closes early in kt+1)
                E = Etiles[kt][h]
                Vs = Vstiles[kt]
                if avp is None:
                    avp = avpA
                for qt in range(qlo, qhi):
                    qi = qt - qoff
                    # start=True zeroes the whole PSUM bank (4 q-tiles):
                    # only the first-emitted head (h1) on kt 0 carries it
                    nc.tensor.matmul(
                        avp[:, qi, ts(h, DH)],
                        lhsT=E[:, ts(qt, P)],
                        rhs=Vs[:, ts(h, DH)],
                        start=(kt == 0 and h == 1 and qi % 4 == 0),
                        stop=(kt == NKT - 1),
                        skip_group_check=True,
                    )

            def vproj_group(g):
                # V projection for k-tiles 4g..4g+3 in one 2-bank psum
                psv = spsum.tile([P, 1024], f32, tag="sco", name="psv")
                for j in range(4):
                    # j=0 / j=2 start=True zeroes banks 0 / 1 of the tile;
                    # j=1 / j=3 accumulate onto the already-zeroed bank
                    for dc in range(NDC):
                        nc.tensor.matmul(
                            psv[:, ts(j, P)],
                            lhsT=vt_s[:, dc, ts(4 * g + j, P)],
                            rhs=wv_s[:, dc, :],
                            start=(j % 2 == 0 and dc == 0),
                            stop=(dc == NDC - 1),
                            skip_group_check=True,
                        )
                # f32 so Pool's normalize_recip can read it directly
                Vf = vfpool.tile([P, 512], f32, tag="vf")
                if c["vf_eng"] == "act":
                    nc.scalar.copy(Vf[:], psv[:, 0:512])
                elif c["vf_eng"] == "dve":
                    nc.vector.tensor_copy(Vf[:], psv[:, 0:512])
                else:
                    nc.scalar.copy(Vf[:, 0:256], psv[:, 0:256])
                    nc.vector.tensor_copy(Vf[:, 256:512], psv[:, 256:512])
                return Vf

            def stats_early(kt):
                # h0 (ACT block) chain wrap-up for kt, run one iteration
                # late so the Pool add feeding it is already stale.  One
                # Pool op does Vs = Vf/D (and leaves 1/D behind).
                if kt < 0:
                    return
                Vf, voff = Vfinfo[kt]
                nc.gpsimd.normalize_recip(
                    Vstiles[kt][:, 0:DH],
                    Vf[:, voff : voff + DH],
                    Dtot[:, 2 * kt : 2 * kt + 1],
                )

            def stats_late(kt):
                # h1 (DVE block) chain: DVE 4x accum pass for D, then Pool
                # normalize_recip for Vs_h1.  The Pool add for h0's accum
                # halves is emitted last so it cannot delay Vs_h1.
                Vf, voff = Vfinfo[kt]
                scr = scrpool.tile([P, L], bf16, tag="scr")
                nc.vector.tensor_scalar(
                    scr[:],
                    Etiles[kt][1][:],
                    1.0,
                    0.0,
                    ALU.mult,
                    ALU.add,
                    accum_out=Dtot[:, 2 * kt + 1 : 2 * kt + 2],
                )
                Vs = vspool.tile([P, P], bf16, tag="vs")
                Vstiles[kt] = Vs
                nc.gpsimd.normalize_recip(
                    Vs[:, DH : 2 * DH],
                    Vf[:, voff + DH : voff + 2 * DH],
                    Dtot[:, 2 * kt + 1 : 2 * kt + 2],
                )
                if kt == 0:
                    # kt0's h0 accum: two quarter slices + one half slice
                    nc.gpsimd.tensor_add(Dq2[:, 0:1], Dq4[:, 0:1], Dq4[:, 1:2])
                    nc.gpsimd.tensor_add(
                        Dtot[:, 0:1], Dq2[:, 0:1], Dsum2[:, 0, 0, 1:2]
                    )
                else:
                    nc.gpsimd.tensor_add(
                        Dtot[:, 2 * kt : 2 * kt + 1],
                        Dsum2[:, kt, 0, 0:1],
                        Dsum2[:, kt, 0, 1:2],
                    )

            def alloc_E(kt):
                E0 = epool.tile([P, L], bf16, tag="E", name=f"E{kt}h0")
                E1 = epool.tile([P, L], bf16, tag="E", name=f"E{kt}h1")
                Etiles[kt] = (E0, E1)

            # warmup: projection spans gated only on their own input
            # chunks; kt0 scores at quarter granularity so the first exps
            # start right after qp0 + the 128-col kp_mini
            qk_proj_span(wq_s, qt_s, QT8, 0, 512, "act")
            qk_proj_span(wk_s, kt_s, KT8, 0, 512, "act")

            for kt in range(NKT):
                alloc_E(kt)
                E0, E1 = Etiles[kt]
                if kt == 0:
                    # first half at quarter granularity (starts on qp0
                    # alone); second half as normal [1024] halves
                    scores_quarter_kt0(0, 0, E0)
                    scores_quarter_kt0(1, 0, E1)
                    qk_proj_span(wq_s, qt_s, QT8, 512, 512, c["warm_qp1"])
                    scores_quarter_kt0(0, 1, E0)
                    scores_quarter_kt0(1, 1, E1)
                    qk_proj_span(wq_s, qt_s, QT8, 1024, 512, c["warm_qp23"])
                    qk_proj_span(wq_s, qt_s, QT8, 1536, 512, c["warm_qp23"])
                    scores_half(0, 0, 1, E0)
                    scores_half(0, 1, 1, E1)
                    Vf0 = vproj_group(0)
                    for j in range(4):
                        Vfinfo[j] = (Vf0, j * P)
                    stats_late(0)
                else:
                    scores_half(kt, 0, 0, E0)
                    scores_half(kt, 1, 0, E1)
                    stats_early(kt - 1)
                    if c["av1_pos"] == "after_early":
                        av_part(kt - 1, 1, 0, 8)
                    if c["vproj_pos"] == "mid" and kt % 4 == 0:
                        Vf = vproj_group(kt // 4)
                        for j in range(4):
                            Vfinfo[kt + j] = (Vf, j * P)
                    scores_half(kt, 0, 1, E0)
                    scores_half(kt, 1, 1, E1)
                    if c["av1_pos"] == "after_f1":
                        av_part(kt - 1, 1, 0, 8)
                    if c["av0_pos"] == "before_kproj":
                        av_part(kt - 1, 0, 0, 8)
                    if kt <= 3:
                        qk_proj_span(wk_s, kt_s, KT8, 512 * kt, 512, c["kproj_eng"])
                    if c["vproj_pos"] == "cur":
                        g, r = divmod(kt + 1, 4)
                        if r == 0 and g < 4:
                            Vf = vproj_group(g)
                            for j in range(4):
                                Vfinfo[4 * g + j] = (Vf, j * P)
                    if c["av0_pos"] == "after_vproj":
                        av_part(kt - 1, 0, 0, 8)
                    stats_late(kt)

            # close kt=15's chains and the in-loop (q-tiles 0-7) AV
            stats_early(NKT - 1)
            av_part(NKT - 1, 1, 0, 8)
            av_part(NKT - 1, 0, 0, 8)

            # tail: replay AV for q-tiles 8-15 into fresh accumulators
            # carved from the (now idle) score-psum pool, so the replay
            # runs concurrently with the qt0-7 evacs instead of waiting
            # for them.  Two SEPARATE tiles (not one 2-bank tile) so the
            # second bank's matmuls carry no false WAR against the first
            # bank's evacuation read.
            avpB0 = spsum.tile([P, 4, P], f32, tag="sco", name="avpB0")
            avpB1 = spsum.tile([P, 4, P], f32, tag="sco", name="avpB1")
            evac_bank(avpA, 0, 0, "act")
            evac_bank(avpA, 1, 4, "dve")
            for kt in range(NKT):
                av_part(kt, 1, 8, 12, avp=avpB0, qoff=8)
                av_part(kt, 0, 8, 12, avp=avpB0, qoff=8)
            evac_bank(avpB0, 0, 8, "act")
            for kt in range(NKT):
                av_part(kt, 1, 12, 16, avp=avpB1, qoff=12)
                av_part(kt, 0, 12, 16, avp=avpB1, qoff=12)
            evac_bank(avpB1, 0, 12, "dve")

    nc.compile()
    return nc


def _get_program():
    if "nc" not in _CACHE:
        _CACHE["nc"] = _build_program()
    return _CACHE["nc"]


def kernel(keys, queries, values, WQ, WK, WV):
    import ml_dtypes

    from concourse import bass_utils

    bf = ml_dtypes.bfloat16
    keys = np.asarray(keys)
    queries = np.asarray(queries)
    values = np.asarray(values)
    WQ = np.asarray(WQ)
    WK = np.asarray(WK)
    WV = np.asarray(WV)

    nc = _get_program()

    in_maps = []
    for cc in range(N_CORES):
        b = cc // 4
        h0 = 2 * (cc % 4)
        h1 = h0 + 1
        in_maps.append(
            {
                "qt": np.ascontiguousarray(queries[b].T).astype(bf),
                "kt": np.ascontiguousarray(keys[b].T).astype(bf),
                "vt": np.ascontiguousarray(values[b].T).astype(bf),
                "wq": np.concatenate([WQ[h0], WQ[h1]], axis=1).astype(bf),
                "wk": np.concatenate([WK[h0], WK[h1]], axis=1).astype(bf),
                "wv": np.concatenate([WV[h0], WV[h1]], axis=1).astype(bf),
            }
        )

    res = bass_utils.run_bass_kernel_spmd(nc, in_maps, core_ids=list(range(N_CORES)))

    out = np.empty((B, L, H * DH), dtype=np.float32)
    for cc in range(N_CORES):
        b = cc // 4
        h0 = 2 * (cc % 4)
        ot = np.asarray(res.results[cc]["out"], dtype=np.float32)  # [128,16,128]
        out[b, :, h0 * DH : (h0 + 2) * DH] = ot.transpose(1, 0, 2).reshape(L, P)
    return out
